# revision 36
# baseline (speedup 1.0000x reference)
"""GAT (graph attention) kernel for 8 Trainium2 NeuronCores.

Contract: kernel(**inputs) takes the FULL inputs of reference.setup_inputs()
and returns the FULL [N, H*F_OUT] float32 output.

Design (v3):
  - dst nodes are partitioned across the 8 cores (12500 each). Edges are
    sorted by dst on the host and routed to the core owning their dst.
  - Phase 1 is REPLICATED: every core computes the full node table
    h = x @ W (plus per-node a_src/a_dst attention scores) for all N nodes
    (~6.8 GFLOP, ~100us on the PE) and writes it to its own DRAM. No
    collectives at all. The DRAM table uses a permuted row order
    (phys = (n%128)*784 + n//128) so the writer emits per-partition
    contiguous full-row runs (BW-bound) instead of 528B descriptors.
  - Table rows are 384 bf16 (768B, %256B as dma_gather requires):
    [h (256) | a_src (4) | a_dst (4) | pad (120)]. Pad cols are never
    read by compute.
  - Phase 2 gathers h[src] per edge with the dma_gather custom ucode
    instruction. Measured on HW this is GPSIMD-Q7-descgen-bound at
    ~5-6ns/index, so the kernel minimizes gathered index count: per-block
    num_idxs is the max over the 8 cores' true counts rounded to 16
    (NOT the global 128-aligned capacity), with un-gathered tail slots
    masked out via dcol == -1 (w forced to 0; z clamped so recycled
    stale slots cannot amplify to inf/NaN). dma_gather constraints:
    int16 indices (edges bucketed by src range, buckets <= 32768 rows)
    and <= 1024 indices per instruction (Q7 scratch limit — >1024
    hard-crashes the device; -1 trailing-trim also crashes, hence the
    mask approach).
  - Phase 2 is software-pipelined two blocks deep: blocks b+1/b+2's
    idx/meta loads and gathers are emitted before block b's compute so
    the in-order per-engine streams overlap gather DMA with DVE/ACT/PE
    work (gath/meta pools hold exactly 3 generations).
  - Per 128-dst block: one-hot M [e,d] built on DVE; MT [d,e] via PE
    transposes of M chunks (psum->SBUF copies alternate ACT/DVE);
    per-edge scores w = max(exp(z), exp(0.2 z)) with
    z = a_src[src] + a_dst[dst] (exact rewrite of exp(leaky_relu(z))),
    weighted segment-sum via PE matmuls psum[d,:] += M_j.T @ [w*h | w],
    normalize by the denominator columns, add bias, write out.
"""

import math

import ml_dtypes
import numpy as np

import concourse.bass as bass
import concourse.tile as tile
from concourse import bacc, mybir

BF16 = mybir.dt.bfloat16
F32 = mybir.dt.float32
I16 = mybir.dt.int16

# problem constants (hardcoded per contract; kernel.py must be self-contained)
N = 100000
E = 3200000
F_IN = 128
F_OUT = 64
HEADS = 4
HF = HEADS * F_OUT  # 256
NEG_SLOPE = 0.2
N_CORES = 8

ROW = 384  # table row in bf16 elems: 256 h | 4 a_src | 4 a_dst | 120 pad
A_SRC = HF  # col offset of a_src
A_DST = HF + HEADS  # col offset of a_dst
USED = HF + 2 * HEADS  # 264 written cols per row
# src-bucket ranges (int16 gather indices need <=32768 rows per bucket;
# sized so each bucket's per-block edge count stays under one 1024-idx
# gather instruction with high probability)
BUCKET_SIZES = [22000, 22000, 22000, 22000, 12352]
MAX_GIDX = 1024  # max indices per dma_gather instruction (Q7 scratch limit)
P1_BATCH = 8  # phase-1 node tiles per DMA


def _host_prep(x, edge_index, W, att_src, att_dst, bias, n_cores):
    """Sort edges by dst, bucket by src range, build per-core inputs."""
    n = x.shape[0]
    n_per_core = n // n_cores
    assert n_per_core * n_cores == n
    blocks = math.ceil(n_per_core / 128)
    n_rows = math.ceil(n / (P1_BATCH * 128)) * P1_BATCH * 128  # table rows
    assert sum(BUCKET_SIZES) == n_rows
    n_buckets = len(BUCKET_SIZES)
    bases = np.zeros(n_buckets + 1, np.int64)
    np.cumsum(BUCKET_SIZES, out=bases[1:])

    W = np.asarray(W, np.float32)
    att_src = np.asarray(att_src, np.float32)
    att_dst = np.asarray(att_dst, np.float32)
    Wh = W.reshape(F_IN, HEADS, F_OUT)
    v_src = np.einsum("khf,hf->kh", Wh, att_src)  # [F_IN, H]
    v_dst = np.einsum("khf,hf->kh", Wh, att_dst)  # [F_IN, H]
    Wv = np.concatenate([W, v_src, v_dst], axis=1)  # [F_IN, 264]
    Wv_bf = Wv.astype(ml_dtypes.bfloat16)

    # Each core gets a ROLLED copy of x so that its own dst rows sit at
    # table rows [0 : blocks*128]: core c's table row i = global node
    # (i + c*n_per_core) % n_rows. This makes the per-block a_dst slice
    # (table[b*128:(b+1)*128]) a static address, identical on every core.
    #
    # The DRAM table additionally uses a PERMUTED row order,
    # phys = (n % 128) * (n_rows // 128) + n // 128, so that the phase-1
    # writer (psum partition p = node chunk*128+p) writes per-partition
    # CONTIGUOUS full-row runs (BW-bound) instead of 528B strided
    # descriptors. All edge src indices are remapped to phys on the host.
    t_rows = n_rows // 128
    xT = np.zeros((F_IN, n_rows), np.float32)
    xT[:, :n] = np.asarray(x, np.float32).T
    xT_bf = xT.astype(ml_dtypes.bfloat16)

    bias_rep = np.broadcast_to(np.asarray(bias, np.float32), (128, HF)).copy()

    src = np.asarray(edge_index[0], np.int64)
    dst = np.asarray(edge_index[1], np.int64)
    order = np.argsort(dst, kind="stable")
    src_s = src[order]
    dst_s = dst[order]
    core_of = dst_s // n_per_core

    # per (core, block, bucket) counts -> global bucket capacities
    per_core = []
    counts = np.zeros((n_cores, blocks, n_buckets), np.int64)
    for c in range(n_cores):
        m = core_of == c
        e_src = (src_s[m] - c * n_per_core) % n_rows  # rolled-table local row
        e_src = (e_src % 128) * t_rows + e_src // 128  # permuted phys row
        dloc = dst_s[m] - c * n_per_core
        blk = dloc // 128
        bkt = np.searchsorted(bases[1:], e_src, side="right")
        np.add.at(counts[c], (blk, bkt), 1)
        per_core.append((e_src, dloc, blk, bkt))
    caps = counts.max(axis=(0, 1))  # [n_buckets]
    caps = np.maximum(128, np.ceil(caps / 128).astype(np.int64) * 128)
    e_blk = int(caps.sum())
    k_ch = e_blk // 128
    boff = np.zeros(n_buckets + 1, np.int64)
    np.cumsum(caps, out=boff[1:])

    # gather instruction splits: (bucket, idx_off_within_block, num_idxs).
    # The SBUF layout uses the global 128-aligned caps, but each BLOCK only
    # gathers its own max-over-cores count (rounded up to 16), which cuts
    # Q7 descriptor-generation work ~15%. Slots past the per-block count
    # keep stale-but-finite tile data and are masked out via dcol == -1.
    cnt_bk = counts.max(axis=0)  # [blocks, n_buckets]
    s_bk = np.minimum(np.ceil(cnt_bk / 16).astype(np.int64) * 16, caps[None, :])
    g_insts = []  # global spans (for idx16 packing)
    for r in range(n_buckets):
        o = 0
        while o < caps[r]:
            s = min(MAX_GIDX, caps[r] - o)
            g_insts.append((r, int(boff[r] + o), int(s)))
            o += s
    g_insts_blk = []  # per-block instruction lists
    for b in range(blocks):
        lst = []
        for r in range(n_buckets):
            o = 0
            while o < s_bk[b, r]:
                s = min(MAX_GIDX, s_bk[b, r] - o)
                lst.append((r, int(boff[r] + o), int(s)))
                o += s
        g_insts_blk.append(lst)

    in_maps = []
    for c in range(n_cores):
        e_src, dloc, blk, bkt = per_core[c]
        # order edges by (block, bucket), then lay out with per-bucket padding
        okey = np.lexsort((bkt, blk))
        e_src, dloc, blk, bkt = e_src[okey], dloc[okey], blk[okey], bkt[okey]
        # slot of each edge inside its (block, bucket) run
        cnt = counts[c]
        starts = np.zeros((blocks, n_buckets), np.int64)
        flat = cnt.reshape(-1)
        np.cumsum(flat[:-1], out=starts.reshape(-1)[1:])
        run_start = starts[blk, bkt]
        within = np.arange(len(e_src)) - run_start
        slot = boff[bkt] + within  # slot within the block's e_blk layout

        # pad slots are -1: the dma_gather ucode trims trailing negative
        # indices BEFORE descriptor generation, so per-core Q7 time tracks
        # the true edge count, not the padded capacity
        idx_loc = np.zeros((blocks, e_blk), np.int16)
        dstloc = np.full((blocks, e_blk), -1.0, np.float32)
        idx_loc[blk, slot] = (e_src - bases[bkt]).astype(np.int16)
        dstloc[blk, slot] = (dloc - blk * 128).astype(np.float32)  # -1 pad rows

        # int16 idx tile layout: per gather instruction i at [i%16, i//16]
        idx16 = np.zeros((blocks, 16, e_blk // 16), np.int16)
        for r, o, s in g_insts:
            sub = idx_loc[:, o : o + s]  # [blocks, s]
            idx16[:, :, o // 16 : (o + s) // 16] = sub.reshape(
                blocks, s // 16, 16
            ).transpose(0, 2, 1)
        idx16 = np.broadcast_to(idx16[:, None, :, :], (blocks, 8, 16, e_blk // 16))
        idx16 = idx16.reshape(blocks, 128, e_blk // 16).copy()

        dcol = dstloc.reshape(blocks, k_ch, 128).transpose(0, 2, 1).copy()
        in_maps.append(
            {
                "xT": np.roll(xT_bf, -c * n_per_core, axis=1),
                "Wv": Wv_bf,
                "bias_rep": bias_rep,
                "idx16": idx16,
                "dcol": dcol,
            }
        )
    params = dict(
        n=n, n_rows=n_rows, n_per_core=n_per_core, blocks=blocks, k_ch=k_ch,
        e_blk=e_blk, g_insts=g_insts, g_insts_blk=g_insts_blk,
        n_buckets=n_buckets,
    )
    return in_maps, params


def _build_program(params, num_devices, iters=1, p1_iters=1, no_gather=False):
    n_rows = params["n_rows"]
    blocks = params["blocks"]
    k_ch = params["k_ch"]
    e_blk = params["e_blk"]
    g_insts = params["g_insts"]
    g_insts_blk = params["g_insts_blk"]
    n_per_core = params["n_per_core"]
    n_tiles = n_rows // 128
    out_pad = blocks * 128

    nc = bacc.Bacc(
        "TRN2",
        target_bir_lowering=False,
        debug=False,
        num_devices=num_devices,
        num_swdge_queues=4,
    )

    xT_d = nc.dram_tensor("xT", [F_IN, n_rows], BF16, kind="ExternalInput")
    Wv_d = nc.dram_tensor("Wv", [F_IN, USED], BF16, kind="ExternalInput")
    bias_d = nc.dram_tensor("bias_rep", [128, HF], F32, kind="ExternalInput")
    idx_d = nc.dram_tensor("idx16", [blocks, 128, e_blk // 16], I16, kind="ExternalInput")
    dcol_d = nc.dram_tensor("dcol", [blocks, 128, k_ch], F32, kind="ExternalInput")
    out_d = nc.dram_tensor("out", [out_pad, HF], F32, kind="ExternalOutput")

    table_d = nc.dram_tensor("table", [n_rows, ROW], BF16)
    bases = np.zeros(len(BUCKET_SIZES) + 1, np.int64)
    np.cumsum(BUCKET_SIZES, out=bases[1:])

    with tile.TileContext(nc) as tc:
        # ---------------- phase 1: node table (replicated on every core) ----
        with (
            tc.tile_pool(name="p1w", bufs=1) as p1w,
            tc.tile_pool(name="p1x", bufs=6) as p1x,
            tc.tile_pool(name="p1s", bufs=4) as p1s,
            tc.tile_pool(name="p1p", bufs=4, space="PSUM") as p1p,
        ):
            wv_t = p1w.tile([128, USED], BF16)
            nc.sync.dma_start(wv_t[:], Wv_d[:, :])
            assert n_tiles % P1_BATCH == 0
            # permuted table: phys row p*t_rows + t holds node t*128 + p, so
            # partition p's P1_BATCH rows per batch are contiguous in DRAM
            # (full 768B rows incl. never-read pad cols -> BW-bound write)
            table_v = table_d[:, :].rearrange("(p t) r -> p t r", p=128)
            for t in [
                tt for _ in range(p1_iters) for tt in range(n_tiles // P1_BATCH)
            ]:
                n0 = t * P1_BATCH * 128
                xt = p1x.tile([128, P1_BATCH * 128], BF16)
                nc.sync.dma_start(xt[:], xT_d[:, n0 : n0 + P1_BATCH * 128])
                st = p1s.tile([128, P1_BATCH, ROW], BF16)
                for q in range(P1_BATCH):
                    ps = p1p.tile([128, USED], F32)
                    nc.tensor.matmul(
                        ps[:], lhsT=xt[:, q * 128 : (q + 1) * 128], rhs=wv_t[:],
                        start=True, stop=True,
                    )
                    if q % 2:
                        nc.scalar.copy(st[:, q, 0:USED], ps[:])
                    else:
                        nc.vector.tensor_copy(st[:, q, 0:USED], ps[:])
                nc.sync.dma_start(
                    table_v[:, t * P1_BATCH : (t + 1) * P1_BATCH, :],
                    st[:],
                )

        # ---------------- phase 2: edge aggregation ----------------
        n_grp = (k_ch + 7) // 8  # MT transpose groups of 8 chunks (1 psum bank)
        with (
            tc.tile_pool(name="cst", bufs=1) as cst,
            tc.tile_pool(name="meta", bufs=3) as meta,
            tc.tile_pool(name="gath", bufs=3) as gath,
            tc.tile_pool(name="onehot", bufs=2) as onehot,
            tc.tile_pool(name="score", bufs=3) as score,
            tc.tile_pool(name="rhsp", bufs=2) as rhsp,
            tc.tile_pool(name="outp", bufs=3) as outp,
            tc.tile_pool(name="psO", bufs=3, space="PSUM") as psO,
            tc.tile_pool(name="psA", bufs=3, space="PSUM") as psA,
            tc.tile_pool(name="psT", bufs=2, space="PSUM") as psT,
        ):
            iota_row_i = cst.tile([128, 128], mybir.dt.int32)
            nc.gpsimd.iota(iota_row_i[:], pattern=[[1, 128]], base=0, channel_multiplier=0)
            iota_row = cst.tile([128, 128], BF16)
            nc.vector.tensor_copy(iota_row[:], iota_row_i[:])
            iota_col_i = cst.tile([128, 1], mybir.dt.int32)
            nc.gpsimd.iota(iota_col_i[:], pattern=[[0, 1]], base=0, channel_multiplier=1)
            iota_col = cst.tile([128, 1], F32)
            nc.vector.tensor_copy(iota_col[:], iota_col_i[:])
            # identity (for PE transpose): iota_row == iota_col
            ident = cst.tile([128, 128], BF16)
            nc.vector.tensor_scalar(
                out=ident[:], in0=iota_row[:], scalar1=iota_col[:, 0:1],
                scalar2=None, op0=mybir.AluOpType.is_equal,
            )
            bias_t = cst.tile([128, HF], F32)
            nc.sync.dma_start(bias_t[:], bias_d[:, :])

            # zero-fill all gather buffers once: trimmed (-1 padded) gather
            # slots leave old tile contents in place, which must be finite
            # (0 * NaN = NaN would poison the masked psum accumulation)
            for _ in range(3):
                g = gath.tile([128, k_ch * ROW], BF16)
                nc.vector.memset(g[:], 0.0)

            def emit_fetch(b):
                """Issue block b's meta loads + gathers (software pipelining:
                called one block ahead so the gather DMA overlaps the previous
                block's compute instead of serializing with it)."""
                idxt = meta.tile([128, e_blk // 16], I16)
                nc.sync.dma_start(idxt[:], idx_d[b, :, :])
                dcol = meta.tile([128, k_ch], F32)
                nc.sync.dma_start(dcol[:], dcol_d[b, :, :])
                # a_dst rows for this block's 128 dst nodes: node b*128+d
                # sits at permuted phys row d*t_rows + b
                adL = meta.tile([128, HEADS], BF16)
                nc.sync.dma_start(
                    adL[:],
                    table_d[:, :]
                    .rearrange("(d t) r -> d t r", d=128)[
                        :, b, A_DST : A_DST + HEADS
                    ],
                )
                g = gath.tile([128, k_ch * ROW], BF16)
                for gi, (r, o, s) in enumerate(
                    [] if no_gather else g_insts_blk[b]
                ):
                    su = (s + 127) // 128 * 128  # out AP covers ceil128
                    nc.gpsimd.dma_gather(
                        out_ap=g[:, (o // 128) * ROW : ((o + su) // 128) * ROW]
                        .rearrange("p (k r) -> p k r", r=ROW),
                        in_ap=table_d[int(bases[r]) : int(bases[r + 1]), :],
                        idxs_ap=idxt[:, o // 16 : (o + s) // 16],
                        num_idxs=s,
                        num_idxs_reg=s,
                        elem_size=ROW,
                        queue_num=(b * len(g_insts) + gi) % 4,
                    )
                return idxt, dcol, adL, g

            bs = [b for _ in range(iters) for b in range(blocks)]
            fetch_q = [emit_fetch(bs[0])]
            if len(bs) > 1:
                fetch_q.append(emit_fetch(bs[1]))
            for bi, b in enumerate(bs):
                if True:
                    dst0 = b * 128
                    idxt, dcol, adL, g = fetch_q.pop(0)
                    if bi + 2 < len(bs):
                        fetch_q.append(emit_fetch(bs[bi + 2]))
                    g3 = g[:].rearrange("p (k r) -> p k r", r=ROW)

                    # one-hot M [e, k*128 d]: per-chunk is_equal against the
                    # per-partition (per-edge) dst slot -> 4x DVE mode
                    M = onehot.tile([128, e_blk], BF16)
                    for j in range(k_ch):
                        nc.vector.tensor_scalar(
                            out=M[:, j * 128 : (j + 1) * 128],
                            in0=iota_row[:],
                            scalar1=dcol[:, j : j + 1],
                            scalar2=None,
                            op0=mybir.AluOpType.is_equal,
                        )
                    # one-hot transpose MT [d, e] via PE transposes of M chunks
                    # (8 chunks per psum bank), psum->SBUF copies alternating
                    # between the ACT and DVE engines
                    MT = onehot.tile([128, e_blk], BF16)
                    for grp in range(n_grp):
                        j0 = grp * 8
                        w = min(8, k_ch - j0)
                        ps_mt = psT.tile([128, 1024], BF16)
                        for jj in range(w):
                            nc.tensor.transpose(
                                ps_mt[:, jj * 128 : (jj + 1) * 128],
                                M[:, (j0 + jj) * 128 : (j0 + jj + 1) * 128],
                                ident[:],
                            )
                        if grp % 2:
                            nc.scalar.copy(
                                MT[:, j0 * 128 : (j0 + w) * 128], ps_mt[:, : w * 128]
                            )
                        else:
                            nc.vector.tensor_copy(
                                MT[:, j0 * 128 : (j0 + w) * 128], ps_mt[:, : w * 128]
                            )
                    # per-edge a_dst: [e, H] = MT_j.T @ adL
                    ps_ad = psA.tile([128, k_ch * HEADS], F32)
                    for j in range(k_ch):
                        nc.tensor.matmul(
                            ps_ad[:, j * HEADS : (j + 1) * HEADS],
                            lhsT=MT[:, j * 128 : (j + 1) * 128],
                            rhs=adL[:],
                            start=True,
                            stop=True,
                        )

                    # scores: z = a_src[src] + a_dst[dst]; w = max(exp z, exp .2z)
                    # z is clamped (stale un-gathered tail slots recycle wb
                    # values) and w is zeroed for invalid slots (dcol == -1)
                    # so stale slots decay to 0 instead of amplifying.
                    z = score.tile([128, k_ch * HEADS], F32)
                    nc.vector.tensor_add(
                        z[:].rearrange("p (k h) -> p k h", h=HEADS),
                        g3[:, :, A_SRC : A_SRC + HEADS],
                        ps_ad[:].rearrange("p (k h) -> p k h", h=HEADS),
                    )
                    nc.vector.tensor_scalar_min(z[:], z[:], 30.0)
                    vmask = score.tile([128, k_ch], BF16)
                    nc.vector.tensor_scalar(
                        out=vmask[:], in0=dcol[:], scalar1=0.0, scalar2=None,
                        op0=mybir.AluOpType.is_ge,
                    )
                    e1 = score.tile([128, k_ch * HEADS], F32)
                    nc.scalar.activation(e1[:], z[:], mybir.ActivationFunctionType.Exp)
                    e2 = score.tile([128, k_ch * HEADS], F32)
                    nc.scalar.activation(
                        e2[:], z[:], mybir.ActivationFunctionType.Exp, scale=NEG_SLOPE
                    )
                    wb = score.tile([128, k_ch * HEADS], BF16)
                    nc.vector.tensor_max(wb[:], e1[:], e2[:])
                    wb4 = wb[:].rearrange("p (k h) -> p k h", h=HEADS)
                    nc.vector.tensor_tensor(
                        out=wb4,
                        in0=wb4,
                        in1=vmask[:].unsqueeze(2).broadcast_to([128, k_ch, HEADS]),
                        op=mybir.AluOpType.mult,
                    )

                    # rhs = [w*h | w] built IN PLACE in g, then the weighted
                    # segment sum psum[d,:] += M_j.T @ rhs_j. The w-broadcast
                    # (ACT), h *= w (DVE) and matmuls (PE) are split into 3
                    # chunk-groups so the three engines pipeline within a
                    # block instead of running as one serial chain.
                    nc.scalar.copy(g3[:, :, A_SRC : A_SRC + HEADS], wb4)
                    wrep = rhsp.tile([128, k_ch * HF], BF16)
                    ps_out = psO.tile([128, HF + HEADS], F32)
                    splits = [0, k_ch // 3, 2 * k_ch // 3, k_ch]
                    for j0, j1 in zip(splits[:-1], splits[1:]):
                        nc.scalar.copy(
                            wrep[:, j0 * HF : j1 * HF].rearrange(
                                "p (k h f) -> p k h f", f=F_OUT, h=HEADS
                            ),
                            wb4[:, j0:j1, :]
                            .unsqueeze(3)
                            .broadcast_to([128, j1 - j0, HEADS, F_OUT]),
                        )
                        nc.vector.tensor_tensor(
                            out=g3[:, j0:j1, 0:HF],
                            in0=g3[:, j0:j1, 0:HF],
                            in1=wrep[:, j0 * HF : j1 * HF].rearrange(
                                "p (k r) -> p k r", r=HF
                            ),
                            op=mybir.AluOpType.mult,
                        )
                        for j in range(j0, j1):
                            nc.tensor.matmul(
                                ps_out[:],
                                lhsT=M[:, j * 128 : (j + 1) * 128],
                                rhs=g3[:, j, 0 : HF + HEADS],
                                start=(j == 0),
                                stop=(j == k_ch - 1),
                            )

                    # normalize + bias
                    den = score.tile([128, HEADS], F32)
                    nc.vector.tensor_scalar_add(den[:], ps_out[:, HF : HF + HEADS], 1e-16)
                    rec = score.tile([128, HEADS], F32)
                    nc.vector.reciprocal(rec[:], den[:])
                    o = outp.tile([128, HF], F32)
                    nc.vector.tensor_tensor(
                        out=o[:].rearrange("p (h f) -> p h f", f=F_OUT),
                        in0=ps_out[:, 0:HF].rearrange("p (h f) -> p h f", f=F_OUT),
                        in1=rec[:].unsqueeze(2).broadcast_to([128, HEADS, F_OUT]),
                        op=mybir.AluOpType.mult,
                    )
                    nc.vector.tensor_add(o[:], o[:], bias_t[:])
                    nc.sync.dma_start(out_d[dst0 : dst0 + 128, :], o[:])

    nc.compile()
    return nc


def _run_pjrt_timed(nc, in_maps, n_cores, reps=5):
    """run_bass_via_pjrt variant that keeps inputs device-resident and times
    repeat executions."""
    import jax
    import time
    from jax.sharding import Mesh, PartitionSpec, NamedSharding
    from jax.experimental.shard_map import shard_map
    from concourse import mybir as mb
    from concourse.bass2jax import (
        _bass_exec_p,
        install_neuronx_cc_hook,
        partition_id_tensor,
    )

    install_neuronx_cc_hook()
    partition_name = nc.partition_id_tensor.name if nc.partition_id_tensor else None
    in_names, out_names, out_avals, zero_outs = [], [], [], []
    for alloc in nc.m.functions[0].allocations:
        if not isinstance(alloc, mb.MemoryLocationSet):
            continue
        name = alloc.memorylocations[0].name
        if alloc.kind == "ExternalInput":
            if name != partition_name:
                in_names.append(name)
        elif alloc.kind == "ExternalOutput":
            out_names.append(name)
            shape = tuple(alloc.tensor_shape)
            dtype = mybir.dt.np(alloc.dtype)
            out_avals.append(jax.core.ShapedArray(shape, dtype))
            zero_outs.append(np.zeros(shape, dtype))
    n_params = len(in_names)
    n_outs = len(out_avals)
    in_names.extend(out_names)
    if partition_name is not None:
        in_names.append(partition_name)
    donate = tuple(range(n_params, n_params + n_outs))

    def _body(*args):
        operands = list(args)
        if partition_name is not None:
            operands.append(partition_id_tensor())
        return tuple(
            _bass_exec_p.bind(
                *operands,
                out_avals=tuple(out_avals),
                in_names=tuple(in_names),
                out_names=tuple(out_names),
                lowering_input_output_aliases=(),
                sim_require_finite=True,
                sim_require_nnan=True,
                nc=nc,
            )
        )

    devices = jax.devices()[:n_cores]
    mesh = Mesh(np.asarray(devices), ("core",))
    spec = PartitionSpec("core")
    sharded = jax.jit(
        shard_map(
            _body,
            mesh=mesh,
            in_specs=(spec,) * (n_params + n_outs),
            out_specs=(spec,) * n_outs,
            check_rep=False,
        ),
        donate_argnums=donate,
        keep_unused=True,
    )
    shd = NamedSharding(mesh, spec)
    in_arrs = [
        jax.device_put(
            np.concatenate([np.asarray(in_maps[c][in_names[i]]) for c in range(n_cores)], axis=0),
            shd,
        )
        for i in range(n_params)
    ]
    out_bufs = [
        jax.device_put(np.zeros((n_cores * z.shape[0], *z.shape[1:]), z.dtype), shd)
        for z in zero_outs
    ]
    times = []
    outs = None
    for r in range(reps):
        t0 = time.perf_counter()
        outs = sharded(*in_arrs, *out_bufs)
        jax.block_until_ready(outs)
        times.append(time.perf_counter() - t0)
        out_bufs = list(outs)
    results = [
        {
            name: np.asarray(outs[i]).reshape(n_cores, *out_avals[i].shape)[c]
            for i, name in enumerate(out_names)
        }
        for c in range(n_cores)
    ]
    return results, times


def run(x, edge_index, W, att_src, att_dst, bias, n_cores=N_CORES, sim=False,
        trace=False, iters=1, reps=5):
    in_maps, params = _host_prep(x, edge_index, W, att_src, att_dst, bias, n_cores)
    nc = _build_program(params, n_cores, iters=iters)
    n_per_core = params["n_per_core"]

    if sim:
        from concourse.bass_interp import MultiCoreSim

        msim = MultiCoreSim(nc, num_cores=n_cores, trace=False)
        for c in range(n_cores):
            msim.cores[c].tensor("table")[:] = 0  # pad cols are never written
            for name, arr in in_maps[c].items():
                msim.cores[c].tensor(name)[:] = arr
        msim.simulate(check_with_hw=False)
        shards = [
            np.asarray(msim.cores[c].tensor("out"))[:n_per_core].astype(np.float32)
            for c in range(n_cores)
        ]
        times = [msim.cores[c].time for c in range(n_cores)]
        return np.concatenate(shards, axis=0), times

    if trace:
        results, times = _run_pjrt_timed(nc, in_maps, n_cores, reps=reps)
        shards = [
            np.asarray(results[c]["out"])[:n_per_core].astype(np.float32)
            for c in range(n_cores)
        ]
        return np.concatenate(shards, axis=0), times

    from concourse.bass_utils import run_bass_kernel_spmd

    res = run_bass_kernel_spmd(nc, in_maps, list(range(n_cores)), trace=False)
    shards = [
        np.asarray(res.results[c]["out"])[:n_per_core].astype(np.float32)
        for c in range(n_cores)
    ]
    return np.concatenate(shards, axis=0), res


def kernel(x, edge_index, W, att_src, att_dst, bias):
    out, _ = run(x, edge_index, W, att_src, att_dst, bias)
    return out



# revision 37
# speedup vs baseline: 1.0035x; 1.0035x over previous
"""GAT (graph attention) kernel for 8 Trainium2 NeuronCores.

Contract: kernel(**inputs) takes the FULL inputs of reference.setup_inputs()
and returns the FULL [N, H*F_OUT] float32 output.

Design (v3):
  - dst nodes are partitioned across the 8 cores (12500 each). Edges are
    sorted by dst on the host and routed to the core owning their dst.
  - Phase 1 is REPLICATED: every core computes the full node table
    h = x @ W (plus per-node a_src/a_dst attention scores) for all N nodes
    (~6.8 GFLOP, ~100us on the PE) and writes it to its own DRAM. No
    collectives at all. The DRAM table uses a permuted row order
    (phys = (n%128)*784 + n//128) so the writer emits per-partition
    contiguous full-row runs (BW-bound) instead of 528B descriptors.
  - Table rows are 384 bf16 (768B, %256B as dma_gather requires):
    [h (256) | a_src (4) | a_dst (4) | pad (120)]. Pad cols are never
    read by compute.
  - Phase 2 gathers h[src] per edge with the dma_gather custom ucode
    instruction. Measured on HW this is GPSIMD-Q7-descgen-bound at
    ~5-6ns/index, so the kernel minimizes gathered index count: per-block
    num_idxs is the max over the 8 cores' true counts rounded to 16
    (NOT the global 128-aligned capacity), with un-gathered tail slots
    masked out via dcol == -1 (w forced to 0; z clamped so recycled
    stale slots cannot amplify to inf/NaN). dma_gather constraints:
    int16 indices (edges bucketed by src range, buckets <= 32768 rows)
    and <= 1024 indices per instruction (Q7 scratch limit — >1024
    hard-crashes the device; -1 trailing-trim also crashes, hence the
    mask approach).
  - Phase 2 is software-pipelined two blocks deep: blocks b+1/b+2's
    idx/meta loads and gathers are emitted before block b's compute so
    the in-order per-engine streams overlap gather DMA with DVE/ACT/PE
    work (gath/meta pools hold exactly 3 generations).
  - Per 128-dst block: one-hot M [e,d] built on DVE; MT [d,e] via PE
    transposes of M chunks (psum->SBUF copies alternate ACT/DVE);
    per-edge scores w = max(exp(z), exp(0.2 z)) with
    z = a_src[src] + a_dst[dst] (exact rewrite of exp(leaky_relu(z))),
    weighted segment-sum via PE matmuls psum[d,:] += M_j.T @ [w*h | w],
    normalize by the denominator columns, add bias, write out.
"""

import math

import ml_dtypes
import numpy as np

import concourse.bass as bass
import concourse.tile as tile
from concourse import bacc, mybir

BF16 = mybir.dt.bfloat16
F32 = mybir.dt.float32
I16 = mybir.dt.int16

# problem constants (hardcoded per contract; kernel.py must be self-contained)
N = 100000
E = 3200000
F_IN = 128
F_OUT = 64
HEADS = 4
HF = HEADS * F_OUT  # 256
NEG_SLOPE = 0.2
N_CORES = 8

ROW = 384  # table row in bf16 elems: 256 h | 4 a_src | 4 a_dst | 120 pad
A_SRC = HF  # col offset of a_src
A_DST = HF + HEADS  # col offset of a_dst
USED = HF + 2 * HEADS  # 264 written cols per row
# src-bucket ranges (int16 gather indices need <=32768 rows per bucket;
# sized so each bucket's per-block edge count stays under one 1024-idx
# gather instruction with high probability)
BUCKET_SIZES = [22000, 22000, 22000, 22000, 12352]
MAX_GIDX = 1024  # max indices per dma_gather instruction (Q7 scratch limit)
P1_BATCH = 8  # phase-1 node tiles per DMA


def _host_prep(x, edge_index, W, att_src, att_dst, bias, n_cores):
    """Sort edges by dst, bucket by src range, build per-core inputs."""
    n = x.shape[0]
    n_per_core = n // n_cores
    assert n_per_core * n_cores == n
    blocks = math.ceil(n_per_core / 128)
    n_rows = math.ceil(n / (P1_BATCH * 128)) * P1_BATCH * 128  # table rows
    assert sum(BUCKET_SIZES) == n_rows
    n_buckets = len(BUCKET_SIZES)
    bases = np.zeros(n_buckets + 1, np.int64)
    np.cumsum(BUCKET_SIZES, out=bases[1:])

    W = np.asarray(W, np.float32)
    att_src = np.asarray(att_src, np.float32)
    att_dst = np.asarray(att_dst, np.float32)
    Wh = W.reshape(F_IN, HEADS, F_OUT)
    v_src = np.einsum("khf,hf->kh", Wh, att_src)  # [F_IN, H]
    v_dst = np.einsum("khf,hf->kh", Wh, att_dst)  # [F_IN, H]
    Wv = np.concatenate([W, v_src, v_dst], axis=1)  # [F_IN, 264]
    Wv_bf = Wv.astype(ml_dtypes.bfloat16)

    # Each core gets a ROLLED copy of x so that its own dst rows sit at
    # table rows [0 : blocks*128]: core c's table row i = global node
    # (i + c*n_per_core) % n_rows. This makes the per-block a_dst slice
    # (table[b*128:(b+1)*128]) a static address, identical on every core.
    #
    # The DRAM table additionally uses a PERMUTED row order,
    # phys = (n % 128) * (n_rows // 128) + n // 128, so that the phase-1
    # writer (psum partition p = node chunk*128+p) writes per-partition
    # CONTIGUOUS full-row runs (BW-bound) instead of 528B strided
    # descriptors. All edge src indices are remapped to phys on the host.
    t_rows = n_rows // 128
    xT = np.zeros((F_IN, n_rows), np.float32)
    xT[:, :n] = np.asarray(x, np.float32).T
    xT_bf = xT.astype(ml_dtypes.bfloat16)

    bias_rep = np.broadcast_to(np.asarray(bias, np.float32), (128, HF)).copy()

    src = np.asarray(edge_index[0], np.int64)
    dst = np.asarray(edge_index[1], np.int64)
    order = np.argsort(dst, kind="stable")
    src_s = src[order]
    dst_s = dst[order]
    core_of = dst_s // n_per_core

    # per (core, block, bucket) counts -> global bucket capacities
    per_core = []
    counts = np.zeros((n_cores, blocks, n_buckets), np.int64)
    for c in range(n_cores):
        m = core_of == c
        e_src = (src_s[m] - c * n_per_core) % n_rows  # rolled-table local row
        e_src = (e_src % 128) * t_rows + e_src // 128  # permuted phys row
        dloc = dst_s[m] - c * n_per_core
        blk = dloc // 128
        bkt = np.searchsorted(bases[1:], e_src, side="right")
        np.add.at(counts[c], (blk, bkt), 1)
        per_core.append((e_src, dloc, blk, bkt))
    caps = counts.max(axis=(0, 1))  # [n_buckets]
    caps = np.maximum(128, np.ceil(caps / 128).astype(np.int64) * 128)
    e_blk = int(caps.sum())
    k_ch = e_blk // 128
    boff = np.zeros(n_buckets + 1, np.int64)
    np.cumsum(caps, out=boff[1:])

    # gather instruction splits: (bucket, idx_off_within_block, num_idxs).
    # The SBUF layout uses the global 128-aligned caps, but each BLOCK only
    # gathers its own max-over-cores count (rounded up to 16), which cuts
    # Q7 descriptor-generation work ~15%. Slots past the per-block count
    # keep stale-but-finite tile data and are masked out via dcol == -1.
    cnt_bk = counts.max(axis=0)  # [blocks, n_buckets]
    s_bk = np.minimum(np.ceil(cnt_bk / 16).astype(np.int64) * 16, caps[None, :])
    g_insts = []  # global spans (for idx16 packing)
    for r in range(n_buckets):
        o = 0
        while o < caps[r]:
            s = min(MAX_GIDX, caps[r] - o)
            g_insts.append((r, int(boff[r] + o), int(s)))
            o += s
    g_insts_blk = []  # per-block instruction lists
    for b in range(blocks):
        lst = []
        for r in range(n_buckets):
            o = 0
            while o < s_bk[b, r]:
                s = min(MAX_GIDX, s_bk[b, r] - o)
                lst.append((r, int(boff[r] + o), int(s)))
                o += s
        g_insts_blk.append(lst)

    in_maps = []
    for c in range(n_cores):
        e_src, dloc, blk, bkt = per_core[c]
        # order edges by (block, bucket), then lay out with per-bucket padding
        okey = np.lexsort((bkt, blk))
        e_src, dloc, blk, bkt = e_src[okey], dloc[okey], blk[okey], bkt[okey]
        # slot of each edge inside its (block, bucket) run
        cnt = counts[c]
        starts = np.zeros((blocks, n_buckets), np.int64)
        flat = cnt.reshape(-1)
        np.cumsum(flat[:-1], out=starts.reshape(-1)[1:])
        run_start = starts[blk, bkt]
        within = np.arange(len(e_src)) - run_start
        slot = boff[bkt] + within  # slot within the block's e_blk layout

        # pad slots are -1: the dma_gather ucode trims trailing negative
        # indices BEFORE descriptor generation, so per-core Q7 time tracks
        # the true edge count, not the padded capacity
        idx_loc = np.zeros((blocks, e_blk), np.int16)
        dstloc = np.full((blocks, e_blk), -1.0, np.float32)
        idx_loc[blk, slot] = (e_src - bases[bkt]).astype(np.int16)
        dstloc[blk, slot] = (dloc - blk * 128).astype(np.float32)  # -1 pad rows

        # int16 idx tile layout: per gather instruction i at [i%16, i//16]
        idx16 = np.zeros((blocks, 16, e_blk // 16), np.int16)
        for r, o, s in g_insts:
            sub = idx_loc[:, o : o + s]  # [blocks, s]
            idx16[:, :, o // 16 : (o + s) // 16] = sub.reshape(
                blocks, s // 16, 16
            ).transpose(0, 2, 1)
        idx16 = np.broadcast_to(idx16[:, None, :, :], (blocks, 8, 16, e_blk // 16))
        idx16 = idx16.reshape(blocks, 128, e_blk // 16).copy()

        dcol = dstloc.reshape(blocks, k_ch, 128).transpose(0, 2, 1).copy()
        in_maps.append(
            {
                "xT": np.roll(xT_bf, -c * n_per_core, axis=1),
                "Wv": Wv_bf,
                "bias_rep": bias_rep,
                "idx16": idx16,
                "dcol": dcol,
            }
        )
    params = dict(
        n=n, n_rows=n_rows, n_per_core=n_per_core, blocks=blocks, k_ch=k_ch,
        e_blk=e_blk, g_insts=g_insts, g_insts_blk=g_insts_blk,
        n_buckets=n_buckets,
    )
    return in_maps, params


def _build_program(params, num_devices, iters=1, p1_iters=1, no_gather=False):
    n_rows = params["n_rows"]
    blocks = params["blocks"]
    k_ch = params["k_ch"]
    e_blk = params["e_blk"]
    g_insts = params["g_insts"]
    g_insts_blk = params["g_insts_blk"]
    n_per_core = params["n_per_core"]
    n_tiles = n_rows // 128
    out_pad = blocks * 128

    nc = bacc.Bacc(
        "TRN2",
        target_bir_lowering=False,
        debug=False,
        num_devices=num_devices,
        num_swdge_queues=4,
    )

    xT_d = nc.dram_tensor("xT", [F_IN, n_rows], BF16, kind="ExternalInput")
    Wv_d = nc.dram_tensor("Wv", [F_IN, USED], BF16, kind="ExternalInput")
    bias_d = nc.dram_tensor("bias_rep", [128, HF], F32, kind="ExternalInput")
    idx_d = nc.dram_tensor("idx16", [blocks, 128, e_blk // 16], I16, kind="ExternalInput")
    dcol_d = nc.dram_tensor("dcol", [blocks, 128, k_ch], F32, kind="ExternalInput")
    out_d = nc.dram_tensor("out", [out_pad, HF], F32, kind="ExternalOutput")

    table_d = nc.dram_tensor("table", [n_rows, ROW], BF16)
    bases = np.zeros(len(BUCKET_SIZES) + 1, np.int64)
    np.cumsum(BUCKET_SIZES, out=bases[1:])

    with tile.TileContext(nc) as tc:
        # ---------------- phase 1: node table (replicated on every core) ----
        with (
            tc.tile_pool(name="p1w", bufs=1) as p1w,
            tc.tile_pool(name="p1x", bufs=6) as p1x,
            tc.tile_pool(name="p1s", bufs=4) as p1s,
            tc.tile_pool(name="p1p", bufs=4, space="PSUM") as p1p,
        ):
            wv_t = p1w.tile([128, USED], BF16)
            nc.sync.dma_start(wv_t[:], Wv_d[:, :])
            assert n_tiles % P1_BATCH == 0
            # permuted table: phys row p*t_rows + t holds node t*128 + p, so
            # partition p's P1_BATCH rows per batch are contiguous in DRAM
            # (full 768B rows incl. never-read pad cols -> BW-bound write)
            table_v = table_d[:, :].rearrange("(p t) r -> p t r", p=128)
            for t in [
                tt for _ in range(p1_iters) for tt in range(n_tiles // P1_BATCH)
            ]:
                n0 = t * P1_BATCH * 128
                xt = p1x.tile([128, P1_BATCH * 128], BF16)
                nc.sync.dma_start(xt[:], xT_d[:, n0 : n0 + P1_BATCH * 128])
                st = p1s.tile([128, P1_BATCH, ROW], BF16)
                for q in range(P1_BATCH):
                    ps = p1p.tile([128, USED], F32)
                    nc.tensor.matmul(
                        ps[:], lhsT=xt[:, q * 128 : (q + 1) * 128], rhs=wv_t[:],
                        start=True, stop=True,
                    )
                    if q % 2:
                        nc.scalar.copy(st[:, q, 0:USED], ps[:])
                    else:
                        nc.vector.tensor_copy(st[:, q, 0:USED], ps[:])
                nc.sync.dma_start(
                    table_v[:, t * P1_BATCH : (t + 1) * P1_BATCH, :],
                    st[:],
                )

        # ---------------- phase 2: edge aggregation ----------------
        n_grp = (k_ch + 7) // 8  # MT transpose groups of 8 chunks (1 psum bank)
        with (
            tc.tile_pool(name="cst", bufs=1) as cst,
            tc.tile_pool(name="meta", bufs=3) as meta,
            tc.tile_pool(name="gath", bufs=3) as gath,
            tc.tile_pool(name="onehot", bufs=2) as onehot,
            tc.tile_pool(name="score", bufs=3) as score,
            tc.tile_pool(name="rhsp", bufs=2) as rhsp,
            tc.tile_pool(name="outp", bufs=3) as outp,
            tc.tile_pool(name="psO", bufs=3, space="PSUM") as psO,
            tc.tile_pool(name="psA", bufs=3, space="PSUM") as psA,
            tc.tile_pool(name="psT", bufs=2, space="PSUM") as psT,
        ):
            iota_row_i = cst.tile([128, 128], mybir.dt.int32)
            nc.gpsimd.iota(iota_row_i[:], pattern=[[1, 128]], base=0, channel_multiplier=0)
            iota_row = cst.tile([128, 128], BF16)
            nc.vector.tensor_copy(iota_row[:], iota_row_i[:])
            iota_col_i = cst.tile([128, 1], mybir.dt.int32)
            nc.gpsimd.iota(iota_col_i[:], pattern=[[0, 1]], base=0, channel_multiplier=1)
            iota_col = cst.tile([128, 1], F32)
            nc.vector.tensor_copy(iota_col[:], iota_col_i[:])
            # identity (for PE transpose): iota_row == iota_col
            ident = cst.tile([128, 128], BF16)
            nc.vector.tensor_scalar(
                out=ident[:], in0=iota_row[:], scalar1=iota_col[:, 0:1],
                scalar2=None, op0=mybir.AluOpType.is_equal,
            )
            bias_t = cst.tile([128, HF], F32)
            nc.sync.dma_start(bias_t[:], bias_d[:, :])

            # zero-fill all gather buffers once: trimmed (-1 padded) gather
            # slots leave old tile contents in place, which must be finite
            # (0 * NaN = NaN would poison the masked psum accumulation)
            for _ in range(3):
                g = gath.tile([128, k_ch * ROW], BF16)
                nc.vector.memset(g[:], 0.0)

            def emit_fetch(b):
                """Issue block b's meta loads + gathers (software pipelining:
                called one block ahead so the gather DMA overlaps the previous
                block's compute instead of serializing with it)."""
                idxt = meta.tile([128, e_blk // 16], I16)
                nc.sync.dma_start(idxt[:], idx_d[b, :, :])
                dcol = meta.tile([128, k_ch], F32)
                nc.sync.dma_start(dcol[:], dcol_d[b, :, :])
                # a_dst rows for this block's 128 dst nodes: node b*128+d
                # sits at permuted phys row d*t_rows + b
                adL = meta.tile([128, HEADS], BF16)
                nc.sync.dma_start(
                    adL[:],
                    table_d[:, :]
                    .rearrange("(d t) r -> d t r", d=128)[
                        :, b, A_DST : A_DST + HEADS
                    ],
                )
                g = gath.tile([128, k_ch * ROW], BF16)
                for gi, (r, o, s) in enumerate(
                    [] if no_gather else g_insts_blk[b]
                ):
                    su = (s + 127) // 128 * 128  # out AP covers ceil128
                    nc.gpsimd.dma_gather(
                        out_ap=g[:, (o // 128) * ROW : ((o + su) // 128) * ROW]
                        .rearrange("p (k r) -> p k r", r=ROW),
                        in_ap=table_d[int(bases[r]) : int(bases[r + 1]), :],
                        idxs_ap=idxt[:, o // 16 : (o + s) // 16],
                        num_idxs=s,
                        num_idxs_reg=s,
                        elem_size=ROW,
                        queue_num=(b * len(g_insts) + gi) % 4,
                    )
                return idxt, dcol, adL, g

            bs = [b for _ in range(iters) for b in range(blocks)]
            fetch_q = [emit_fetch(bs[0])]
            if len(bs) > 1:
                fetch_q.append(emit_fetch(bs[1]))
            for bi, b in enumerate(bs):
                if True:
                    dst0 = b * 128
                    idxt, dcol, adL, g = fetch_q.pop(0)
                    if bi + 2 < len(bs):
                        fetch_q.append(emit_fetch(bs[bi + 2]))
                    g3 = g[:].rearrange("p (k r) -> p k r", r=ROW)

                    # one-hot M [e, k*128 d]: per-chunk is_equal against the
                    # per-partition (per-edge) dst slot -> 4x DVE mode
                    M = onehot.tile([128, e_blk], BF16)
                    for j in range(k_ch):
                        nc.vector.tensor_scalar(
                            out=M[:, j * 128 : (j + 1) * 128],
                            in0=iota_row[:],
                            scalar1=dcol[:, j : j + 1],
                            scalar2=None,
                            op0=mybir.AluOpType.is_equal,
                        )
                    # one-hot transpose MT [d, e] via PE transposes of M chunks
                    # (8 chunks per psum bank), psum->SBUF copies alternating
                    # between the ACT and DVE engines
                    MT = onehot.tile([128, e_blk], BF16)
                    for grp in range(n_grp):
                        j0 = grp * 8
                        w = min(8, k_ch - j0)
                        ps_mt = psT.tile([128, 1024], BF16)
                        for jj in range(w):
                            nc.tensor.transpose(
                                ps_mt[:, jj * 128 : (jj + 1) * 128],
                                M[:, (j0 + jj) * 128 : (j0 + jj + 1) * 128],
                                ident[:],
                            )
                        if grp % 2:
                            nc.scalar.copy(
                                MT[:, j0 * 128 : (j0 + w) * 128], ps_mt[:, : w * 128]
                            )
                        else:
                            nc.vector.tensor_copy(
                                MT[:, j0 * 128 : (j0 + w) * 128], ps_mt[:, : w * 128]
                            )
                    # per-edge a_dst: [e, H] = MT_j.T @ adL
                    ps_ad = psA.tile([128, k_ch * HEADS], F32)
                    for j in range(k_ch):
                        nc.tensor.matmul(
                            ps_ad[:, j * HEADS : (j + 1) * HEADS],
                            lhsT=MT[:, j * 128 : (j + 1) * 128],
                            rhs=adL[:],
                            start=True,
                            stop=True,
                        )

                    # scores: z = a_src[src] + a_dst[dst]; w = max(exp z, exp .2z)
                    # z is clamped (stale un-gathered tail slots recycle wb
                    # values) and w is zeroed for invalid slots (dcol == -1)
                    # so stale slots decay to 0 instead of amplifying.
                    z = score.tile([128, k_ch * HEADS], F32)
                    nc.vector.tensor_add(
                        z[:].rearrange("p (k h) -> p k h", h=HEADS),
                        g3[:, :, A_SRC : A_SRC + HEADS],
                        ps_ad[:].rearrange("p (k h) -> p k h", h=HEADS),
                    )
                    nc.vector.tensor_scalar_min(z[:], z[:], 30.0)
                    vmask = score.tile([128, k_ch], BF16)
                    nc.vector.tensor_scalar(
                        out=vmask[:], in0=dcol[:], scalar1=0.0, scalar2=None,
                        op0=mybir.AluOpType.is_ge,
                    )
                    e1 = score.tile([128, k_ch * HEADS], F32)
                    nc.scalar.activation(e1[:], z[:], mybir.ActivationFunctionType.Exp)
                    e2 = score.tile([128, k_ch * HEADS], F32)
                    nc.scalar.activation(
                        e2[:], z[:], mybir.ActivationFunctionType.Exp, scale=NEG_SLOPE
                    )
                    wb = score.tile([128, k_ch * HEADS], BF16)
                    nc.vector.tensor_max(wb[:], e1[:], e2[:])
                    wb4 = wb[:].rearrange("p (k h) -> p k h", h=HEADS)
                    nc.vector.tensor_tensor(
                        out=wb4,
                        in0=wb4,
                        in1=vmask[:].unsqueeze(2).broadcast_to([128, k_ch, HEADS]),
                        op=mybir.AluOpType.mult,
                    )

                    # rhs = [w*h | w] built IN PLACE in g, then the weighted
                    # segment sum psum[d,:] += M_j.T @ rhs_j. The w-broadcast
                    # (ACT), h *= w (DVE) and matmuls (PE) are split into 3
                    # chunk-groups so the three engines pipeline within a
                    # block instead of running as one serial chain.
                    nc.scalar.copy(g3[:, :, A_SRC : A_SRC + HEADS], wb4)
                    wrep = rhsp.tile([128, k_ch * HF], BF16)
                    ps_out = psO.tile([128, HF + HEADS], F32)
                    splits = [i * k_ch // 5 for i in range(5)] + [k_ch]
                    for j0, j1 in zip(splits[:-1], splits[1:]):
                        nc.scalar.copy(
                            wrep[:, j0 * HF : j1 * HF].rearrange(
                                "p (k h f) -> p k h f", f=F_OUT, h=HEADS
                            ),
                            wb4[:, j0:j1, :]
                            .unsqueeze(3)
                            .broadcast_to([128, j1 - j0, HEADS, F_OUT]),
                        )
                        nc.vector.tensor_tensor(
                            out=g3[:, j0:j1, 0:HF],
                            in0=g3[:, j0:j1, 0:HF],
                            in1=wrep[:, j0 * HF : j1 * HF].rearrange(
                                "p (k r) -> p k r", r=HF
                            ),
                            op=mybir.AluOpType.mult,
                        )
                        for j in range(j0, j1):
                            nc.tensor.matmul(
                                ps_out[:],
                                lhsT=M[:, j * 128 : (j + 1) * 128],
                                rhs=g3[:, j, 0 : HF + HEADS],
                                start=(j == 0),
                                stop=(j == k_ch - 1),
                            )

                    # normalize + bias
                    den = score.tile([128, HEADS], F32)
                    nc.vector.tensor_scalar_add(den[:], ps_out[:, HF : HF + HEADS], 1e-16)
                    rec = score.tile([128, HEADS], F32)
                    nc.vector.reciprocal(rec[:], den[:])
                    o = outp.tile([128, HF], F32)
                    nc.vector.tensor_tensor(
                        out=o[:].rearrange("p (h f) -> p h f", f=F_OUT),
                        in0=ps_out[:, 0:HF].rearrange("p (h f) -> p h f", f=F_OUT),
                        in1=rec[:].unsqueeze(2).broadcast_to([128, HEADS, F_OUT]),
                        op=mybir.AluOpType.mult,
                    )
                    nc.vector.tensor_add(o[:], o[:], bias_t[:])
                    nc.sync.dma_start(out_d[dst0 : dst0 + 128, :], o[:])

    nc.compile()
    return nc


def _run_pjrt_timed(nc, in_maps, n_cores, reps=5):
    """run_bass_via_pjrt variant that keeps inputs device-resident and times
    repeat executions."""
    import jax
    import time
    from jax.sharding import Mesh, PartitionSpec, NamedSharding
    from jax.experimental.shard_map import shard_map
    from concourse import mybir as mb
    from concourse.bass2jax import (
        _bass_exec_p,
        install_neuronx_cc_hook,
        partition_id_tensor,
    )

    install_neuronx_cc_hook()
    partition_name = nc.partition_id_tensor.name if nc.partition_id_tensor else None
    in_names, out_names, out_avals, zero_outs = [], [], [], []
    for alloc in nc.m.functions[0].allocations:
        if not isinstance(alloc, mb.MemoryLocationSet):
            continue
        name = alloc.memorylocations[0].name
        if alloc.kind == "ExternalInput":
            if name != partition_name:
                in_names.append(name)
        elif alloc.kind == "ExternalOutput":
            out_names.append(name)
            shape = tuple(alloc.tensor_shape)
            dtype = mybir.dt.np(alloc.dtype)
            out_avals.append(jax.core.ShapedArray(shape, dtype))
            zero_outs.append(np.zeros(shape, dtype))
    n_params = len(in_names)
    n_outs = len(out_avals)
    in_names.extend(out_names)
    if partition_name is not None:
        in_names.append(partition_name)
    donate = tuple(range(n_params, n_params + n_outs))

    def _body(*args):
        operands = list(args)
        if partition_name is not None:
            operands.append(partition_id_tensor())
        return tuple(
            _bass_exec_p.bind(
                *operands,
                out_avals=tuple(out_avals),
                in_names=tuple(in_names),
                out_names=tuple(out_names),
                lowering_input_output_aliases=(),
                sim_require_finite=True,
                sim_require_nnan=True,
                nc=nc,
            )
        )

    devices = jax.devices()[:n_cores]
    mesh = Mesh(np.asarray(devices), ("core",))
    spec = PartitionSpec("core")
    sharded = jax.jit(
        shard_map(
            _body,
            mesh=mesh,
            in_specs=(spec,) * (n_params + n_outs),
            out_specs=(spec,) * n_outs,
            check_rep=False,
        ),
        donate_argnums=donate,
        keep_unused=True,
    )
    shd = NamedSharding(mesh, spec)
    in_arrs = [
        jax.device_put(
            np.concatenate([np.asarray(in_maps[c][in_names[i]]) for c in range(n_cores)], axis=0),
            shd,
        )
        for i in range(n_params)
    ]
    out_bufs = [
        jax.device_put(np.zeros((n_cores * z.shape[0], *z.shape[1:]), z.dtype), shd)
        for z in zero_outs
    ]
    times = []
    outs = None
    for r in range(reps):
        t0 = time.perf_counter()
        outs = sharded(*in_arrs, *out_bufs)
        jax.block_until_ready(outs)
        times.append(time.perf_counter() - t0)
        out_bufs = list(outs)
    results = [
        {
            name: np.asarray(outs[i]).reshape(n_cores, *out_avals[i].shape)[c]
            for i, name in enumerate(out_names)
        }
        for c in range(n_cores)
    ]
    return results, times


def run(x, edge_index, W, att_src, att_dst, bias, n_cores=N_CORES, sim=False,
        trace=False, iters=1, reps=5):
    in_maps, params = _host_prep(x, edge_index, W, att_src, att_dst, bias, n_cores)
    nc = _build_program(params, n_cores, iters=iters)
    n_per_core = params["n_per_core"]

    if sim:
        from concourse.bass_interp import MultiCoreSim

        msim = MultiCoreSim(nc, num_cores=n_cores, trace=False)
        for c in range(n_cores):
            msim.cores[c].tensor("table")[:] = 0  # pad cols are never written
            for name, arr in in_maps[c].items():
                msim.cores[c].tensor(name)[:] = arr
        msim.simulate(check_with_hw=False)
        shards = [
            np.asarray(msim.cores[c].tensor("out"))[:n_per_core].astype(np.float32)
            for c in range(n_cores)
        ]
        times = [msim.cores[c].time for c in range(n_cores)]
        return np.concatenate(shards, axis=0), times

    if trace:
        results, times = _run_pjrt_timed(nc, in_maps, n_cores, reps=reps)
        shards = [
            np.asarray(results[c]["out"])[:n_per_core].astype(np.float32)
            for c in range(n_cores)
        ]
        return np.concatenate(shards, axis=0), times

    from concourse.bass_utils import run_bass_kernel_spmd

    res = run_bass_kernel_spmd(nc, in_maps, list(range(n_cores)), trace=False)
    shards = [
        np.asarray(res.results[c]["out"])[:n_per_core].astype(np.float32)
        for c in range(n_cores)
    ]
    return np.concatenate(shards, axis=0), res


def kernel(x, edge_index, W, att_src, att_dst, bias):
    out, _ = run(x, edge_index, W, att_src, att_dst, bias)
    return out

